# revision 37
# baseline (speedup 1.0000x reference)
"""Trainium2 Bass kernel for agent attention (sparse_attention problem).

Per-core work (data-parallel over batch B=8 across 8 NeuronCores):
  x[b] [256, 64, 64] -> qkv 3x3 conv (dif-conv + BN folded into weights)
  -> agent attention (8 heads, d=32, 64 agent tokens)
  -> depthwise 3x3 pe conv on v -> 1x1 proj.

Fully pipelined single phase: v conv -> agent tokens computed directly
from 8x8 block sums of x (so stage-1 needs only k) -> k conv with
stage-1 attention interleaved per row-block pair -> q conv with stage-2
+ proj interleaved (lagged one pair). All matmuls bf16 (fp32 PSUM).
"""
import numpy as np

NUM_HEADS = 8
AGENT_NUM = 64
THETA = 0.7
C = 256
H = W = 64
HW = H * W
D = C // NUM_HEADS          # 32
N_AG = AGENT_NUM            # 64
PS = 8                      # pool size
N_CORES = 8
B = 8

_cache = {}


def _build():
    import concourse.bass as bass
    import concourse.tile as tile
    from concourse import bacc, mybir

    f32 = mybir.dt.float32
    f32r = mybir.dt.float32r
    bf16 = mybir.dt.bfloat16
    AF = mybir.ActivationFunctionType
    ALU = mybir.AluOpType
    AX = mybir.AxisListType

    nc = bacc.Bacc("TRN2", target_bir_lowering=False, debug=False,
                   enable_asserts=True, num_devices=N_CORES)

    X = nc.dram_tensor("x", [2, 128, H, W], bf16, kind="ExternalInput").ap()
    WQ = nc.dram_tensor("wq", [6, 128, 2, 9, 128], bf16,
                        kind="ExternalInput").ap()
    AWF = nc.dram_tensor("awf", [2, 128, 2, 9, 128], f32r,
                         kind="ExternalInput").ap()
    BQ = nc.dram_tensor("bq", [128, 6], f32, kind="ExternalInput").ap()
    PEW = nc.dram_tensor("pew", [128, 2, 9], f32, kind="ExternalInput").ap()
    PW = nc.dram_tensor("pw", [128, 2 * 256], bf16, kind="ExternalInput").ap()
    PB = nc.dram_tensor("pb", [128, 2], f32, kind="ExternalInput").ap()
    IDN = nc.dram_tensor("idn", [128, 128], bf16, kind="ExternalInput").ap()
    OUT = nc.dram_tensor("out", [2, 128, HW], f32, kind="ExternalOutput").ap()

    # softmax exp scale: d^-0.5, with the 1/64 agent-pool mean folded in
    SCALE = (D ** -0.5) / (PS * PS)

    with tile.TileContext(nc) as tc:
        from contextlib import ExitStack
        with ExitStack() as top:
            pers = top.enter_context(tc.tile_pool(name="pers", bufs=1))
            q_sb = [pers.tile([128, HW], bf16, tag=f"q{i}", name=f"q{i}")
                    for i in range(2)]
            k_sb = [pers.tile([128, HW], bf16, tag=f"k{i}", name=f"k{i}")
                    for i in range(2)]
            v_pad = [pers.tile([128, 66 * 66], bf16, tag=f"vp{i}",
                               name=f"vp{i}") for i in range(2)]
            att_out = [pers.tile([128, HW], bf16, tag=f"ao{i}", name=f"ao{i}")
                       for i in range(2)]
            bq = pers.tile([128, 6], f32, tag="bq", name="bq")
            idn = pers.tile([128, 128], bf16, tag="idn", name="idn")
            pew = pers.tile([128, 2, 9], f32, tag="pew", name="pew")
            ab64 = pers.tile([128, 2], f32, tag="ab64", name="ab64")
            abd_t = pers.tile([128, 512], bf16, tag="abd", name="abd")
            a_bd4 = [abd_t[:, 256 * i:256 * (i + 1)] for i in range(2)]
            az_t = pers.tile([128, 4 * 68], bf16, tag="az", name="az")
            attnZ = [az_t[:, 68 * i:68 * (i + 1)] for i in range(4)]
            pw = pers.tile([128, 2 * 256], bf16, tag="pw", name="pwt")
            pb = pers.tile([128, 2], f32, tag="pb", name="pbt")
            nc.sync.dma_start(bq[:], BQ[:])
            nc.sync.dma_start(idn[:], IDN[:])
            nc.sync.dma_start(pew[:], PEW[:])
            nc.sync.dma_start(pw[:], PW[:])
            nc.sync.dma_start(pb[:], PB[:])
            pwv = pw[:].rearrange("p (a b) -> p a b", a=2, b=256)

            for cc in range(2):
                vv = v_pad[cc][:].rearrange("p (r c) -> p r c", r=66, c=66)
                nc.vector.memset(vv[:, 0:1, :], 0.0)
                nc.vector.memset(vv[:, 65:66, :], 0.0)
                nc.vector.memset(vv[:, :, 0:1], 0.0)
                nc.vector.memset(vv[:, :, 65:66], 0.0)

            s1sb = top.enter_context(tc.tile_pool(name="s1sb", bufs=2))

            with ExitStack() as ph:
                cpool = ph.enter_context(tc.tile_pool(name="conv", bufs=1))
                wpool = ph.enter_context(tc.tile_pool(name="wq", bufs=1))
                cps = ph.enter_context(
                    tc.tile_pool(name="cps", bufs=2, space="PSUM"))
                pepool = ph.enter_context(tc.tile_pool(name="pep", bufs=2))
                xbpool = ph.enter_context(tc.tile_pool(name="xbp", bufs=2))

                x_pad = [cpool.tile([128, 66 * 66], bf16, tag=f"xp{i}",
                                    name=f"xp{i}") for i in range(2)]
                x_pk = [cpool.tile([128, HW], bf16, tag=f"xk{i}",
                                   name=f"xk{i}") for i in range(2)]
                # packed contiguous DMA (few descriptors), then pad
                # on-chip with fast bf16 copies; first halves of both kc
                # and the v-conv weights go first so rb 0-2 start early
                for kc in range(2):
                    nc.sync.dma_start(x_pk[kc][:, 0:2048], X[kc, :, 0:32])

                def load_w(mc):
                    wts = []
                    for kc in range(2):
                        wt = wpool.tile([128, 9, 128], bf16, tag="w",
                                        name="w", bufs=4)
                        nc.sync.dma_start(wt[:], WQ[mc, :, kc])
                        wts.append(wt)
                    return wts

                wv = [load_w(4), load_w(5)]
                for kc in range(2):
                    nc.sync.dma_start(x_pk[kc][:, 2048:4096],
                                      X[kc, :, 32:64])
                for kc in range(2):
                    xv = x_pad[kc][:].rearrange(
                        "p (r c) -> p r c", r=66, c=66)
                    nc.vector.memset(xv[:, 0:1, :], 0.0)
                    nc.vector.memset(xv[:, 65:66, :], 0.0)
                    nc.vector.memset(xv[:, :, 0:1], 0.0)
                    nc.vector.memset(xv[:, :, 65:66], 0.0)
                for kc in range(2):
                    xv = x_pad[kc][:].rearrange(
                        "p (r c) -> p r c", r=66, c=66)
                    xkv = x_pk[kc][:].rearrange(
                        "p (r c) -> p r c", r=64, c=64)
                    nc.vector.tensor_copy(xv[:, 1:33, 1:65], xkv[:, 0:32])
                for kc in range(2):
                    xv = x_pad[kc][:].rearrange(
                        "p (r c) -> p r c", r=66, c=66)
                    xkv = x_pk[kc][:].rearrange(
                        "p (r c) -> p r c", r=64, c=64)
                    nc.vector.tensor_copy(xv[:, 33:65, 1:65], xkv[:, 32:64])

                nc.vector.tensor_scalar_mul(ab64[:], bq[:, 0:2], 64.0)

                # 8x8 block sums of padded x for all 9 conv offsets:
                # xblk[kc][cin, 3*ky+kx, 8*by+bx], kept in f32
                xblk_b = []
                for kc in range(2):
                    xv = x_pad[kc][:].rearrange(
                        "p (r c) -> p r c", r=66, c=66)
                    rs = xbpool.tile([128, 3, 66, 8], f32, tag="rs",
                                     name="rs")
                    for kx in range(3):
                        nc.vector.tensor_reduce(
                            rs[:, kx], xv[:, :, kx:kx + 64].rearrange(
                                "p r (b d) -> p r b d", b=8, d=8),
                            AX.X, ALU.add)
                    xbf = xbpool.tile([128, 9, 64], f32r, tag="xbf",
                                      name="xbf")
                    with nc.allow_low_precision(
                            reason="f32r write of f32-accumulated sums"):
                        for ky in range(3):
                            for kx in range(3):
                                nc.vector.tensor_reduce(
                                    xbf[:, 3 * ky + kx].rearrange(
                                        "p (y x) -> p y x", y=8, x=8),
                                    rs[:, kx, ky:ky + 64, :].rearrange(
                                        "p (b d) x -> p b x d", b=8, d=8),
                                    AX.X, ALU.add)
                    xblk_b.append(xbf)

                def conv_rb(mc, wts, rb):
                    ps_t = cps.tile([128, 512], f32, tag="cps", name="cpst")
                    psv = ps_t[:].rearrange("p (r c) -> p r c", r=8, c=64)
                    i = 0
                    for kc in range(2):
                        xv = x_pad[kc][:].rearrange(
                            "p (r c) -> p r c", r=66, c=66)
                        for s in range(9):
                            ky, kx = s // 3, s % 3
                            rhs = xv[:, 8 * rb + ky: 8 * rb + ky + 8,
                                     kx: kx + 64]
                            nc.tensor.matmul(
                                psv, wts[kc][:, s, :], rhs,
                                start=(i == 0), stop=(i == 17))
                            i += 1
                    bias = bq[:, mc: mc + 1]
                    if mc < 2:
                        dst = q_sb[mc][:, 512 * rb: 512 * (rb + 1)]
                        nc.scalar.activation(dst, ps_t[:], AF.Identity,
                                             bias=bias)
                    elif mc < 4:
                        dst = k_sb[mc - 2][:, 512 * rb: 512 * (rb + 1)]
                        nc.scalar.activation(dst, ps_t[:], AF.Identity,
                                             bias=bias)
                    else:
                        vv = v_pad[mc - 4][:].rearrange(
                            "p (r c) -> p r c", r=66, c=66)
                        dst = vv[:, 8 * rb + 1: 8 * rb + 9, 1:65]
                        nc.scalar.activation(dst, psv, AF.Identity,
                                             bias=bias)

                # pe depthwise conv on DVE in bf16: scaled-copy taps via
                # tensor_scalar_mul (4x mode) + tensor_tensor adds (2x mode)
                def pe_conv(cc):
                    vvf = v_pad[cc][:].rearrange(
                        "p (r c) -> p r c", r=66, c=66)
                    dst = att_out[cc][:].rearrange(
                        "p (r c) -> p r c", r=64, c=64)
                    for s in range(9):
                        ky, kx = s // 3, s % 3
                        sv = vvf[:, ky: ky + 64, kx: kx + 64]
                        if s == 0:
                            nc.vector.tensor_scalar_mul(
                                dst, sv, pew[:, cc, 0:1])
                        else:
                            tmp = pepool.tile([128, HW], bf16, tag="pet",
                                              name="pet")
                            tv = tmp[:].rearrange(
                                "p (r c) -> p r c", r=64, c=64)
                            nc.vector.tensor_scalar_mul(
                                tv, sv, pew[:, cc, s:s + 1])
                            nc.vector.tensor_tensor(dst, tv, dst, ALU.add)

                # ---- v convs + agent tokens + transposed v ----
                vts = [None] * 32
                # q-group weights (bf16 for the conv, f32 for the
                # agent-token matmuls) — DMA'd after the v weights so the
                # first conv isn't queued behind them
                aw = []
                awf = []
                for mc in range(2):
                    awm = []
                    awfm = []
                    for kc in range(2):
                        wt = wpool.tile([128, 9, 128], bf16, tag="aw",
                                        name="aw", bufs=4)
                        nc.sync.dma_start(wt[:], WQ[mc, :, kc])
                        awm.append(wt)
                        wtf = wpool.tile([128, 9, 128], f32r, tag="awf",
                                         name="awf", bufs=4)
                        nc.sync.dma_start(wtf[:], AWF[mc, :, kc])
                        awfm.append(wtf)
                    aw.append(awm)
                    awf.append(awfm)
                for rb in range(8):
                    conv_rb(4, wv[0], rb)
                with ExitStack() as vph:
                    a_pp = vph.enter_context(
                        tc.tile_pool(name="aps", bufs=1, space="PSUM"))
                    tr_ps = vph.enter_context(
                        tc.tile_pool(name="trps", bufs=2, space="PSUM"))
                    a_ps = a_pp.tile([128, 128], f32, tag="ap", name="apt")
                    for mc in range(2):
                        i = 0
                        for kc in range(2):
                            for s in range(9):
                                nc.tensor.matmul(
                                    a_ps[:, 64 * mc:64 * (mc + 1)],
                                    awf[mc][kc][:, s, :],
                                    xblk_b[kc][:, s, :],
                                    start=(i == 0), stop=(i == 17),
                                    skip_group_check=True)
                                i += 1

                    def make_vt(ch):
                        vtc = s1sb.tile([128, 264], bf16, tag="vt",
                                        name="vt", bufs=32)
                        vts[ch] = vtc
                        vtv = vtc[:].rearrange("p (a b) -> p a b", a=4, b=66)
                        nc.vector.memset(vtv[:, :, 64:66], 1.0)
                        for cc in range(2):
                            vv = v_pad[cc][:].rearrange(
                                "p (r c) -> p r c", r=66, c=66)
                            vstg = s1sb.tile([128, 128], bf16, tag="vstg",
                                             name="vstg")
                            nc.vector.tensor_copy(
                                vstg[:].rearrange(
                                    "p (r c) -> p r c", r=2, c=64),
                                vv[:, 2 * ch + 1: 2 * ch + 3, 1:65])
                            tp = tr_ps.tile([128, 128], bf16, tag="tr",
                                            name="trt")
                            nc.tensor.transpose(tp[:], vstg[:], idn[:])
                            nc.vector.tensor_copy(
                                vtc[:].rearrange(
                                    "p (a b) -> p a b", a=4, b=66)[
                                    :, 2 * cc: 2 * cc + 2, 0:64],
                                tp[:].rearrange("p (a b) -> p a b",
                                                a=2, b=64))

                    for rb in range(8):
                        conv_rb(5, wv[1], rb)
                        for ch in range(4 * rb, 4 * rb + 4):
                            make_vt(ch)
                    # agent tokens: block-diag a (+64*bias), bf16
                    for cc in range(2):
                        nc.vector.memset(a_bd4[cc], 0.0)
                        for j in range(4):
                            nc.vector.tensor_scalar_add(
                                a_bd4[cc][32 * j:32 * j + 32,
                                          64 * j:64 * j + 64],
                                a_ps[32 * j:32 * j + 32,
                                     64 * cc:64 * (cc + 1)],
                                ab64[32 * j:32 * j + 32, cc:cc + 1])
                pe_conv(0)
                pe_conv(1)

                # ---- k convs + stage 1 ----
                with ExitStack() as s1ph:
                    st_ps = s1ph.enter_context(
                        tc.tile_pool(name="stps", bufs=2, space="PSUM"))
                    at_pp = s1ph.enter_context(
                        tc.tile_pool(name="atps", bufs=1, space="PSUM"))
                    # interleaved long-lived accumulation groups must each
                    # own a PSUM bank
                    attn_ps = [at_pp.tile([128, 66], f32, tag=f"at{i}",
                                          name=f"at{i}")[:] for i in range(4)]

                    ets = [None] * 32

                    def s1_sp(ch):
                        sp = st_ps.tile([128, 512], f32, tag="st",
                                        name="stt")
                        for cc in range(2):
                            nc.tensor.matmul(
                                sp[:, 256 * cc:256 * (cc + 1)],
                                k_sb[cc][:, 128 * ch:128 * (ch + 1)],
                                a_bd4[cc][:], start=True, stop=True,
                                skip_group_check=True)
                        et = s1sb.tile([128, 512], bf16, tag="et", name="et",
                                       bufs=4)
                        nc.scalar.activation(et[:], sp[:], AF.Exp,
                                             scale=SCALE)
                        ets[ch] = et

                    def s1_agg(ch):
                        for hp in range(4):
                            nc.tensor.matmul(
                                attn_ps[hp],
                                ets[ch][:, 128 * hp:128 * (hp + 1)],
                                vts[ch][:, 66 * hp:66 * hp + 66],
                                start=(ch == 0), stop=(ch == 31))

                    wk = [load_w(2), load_w(3)]
                    for r in range(8):
                        conv_rb(2, wk[0], r)
                        conv_rb(3, wk[1], r)
                        for ch in range(4 * r, 4 * r + 4):
                            s1_sp(ch)
                            if ch > 0:
                                s1_agg(ch - 1)
                    s1_agg(31)

                    # normalize stage-1 rows by Z1, build attnZ (+ones cols)
                    for hp in range(4):
                        r1 = s1sb.tile([128, 1], f32, tag="r1", name="r1")
                        nc.vector.reciprocal(r1[:], attn_ps[hp][:, 64:65])
                        nc.vector.memset(attnZ[hp], 0.0)
                        nc.vector.memset(attnZ[hp][0:64, 64:65], 1.0)
                        nc.vector.memset(attnZ[hp][64:128, 65:66], 1.0)
                        nc.vector.tensor_scalar_mul(
                            attnZ[hp][0:64, 0:32], attn_ps[hp][0:64, 0:32],
                            r1[0:64, :])
                        nc.vector.tensor_scalar_mul(
                            attnZ[hp][64:128, 32:64],
                            attn_ps[hp][64:128, 32:64], r1[64:128, :])

                # ---- q convs + stage 2 + proj ----
                with ExitStack() as s2ph:
                    s2sb = s2ph.enter_context(
                        tc.tile_pool(name="s2sb", bufs=3))
                    osb = s2ph.enter_context(tc.tile_pool(name="osb",
                                                          bufs=3))
                    s2_ps = s2ph.enter_context(
                        tc.tile_pool(name="s2ps", bufs=2, space="PSUM"))
                    g_ps = s2ph.enter_context(
                        tc.tile_pool(name="gps", bufs=2, space="PSUM"))
                    t_ps = s2ph.enter_context(
                        tc.tile_pool(name="tps", bufs=2, space="PSUM"))

                    def stage2(nt):
                        for cc in range(2):
                            tp = t_ps.tile([128, 512], f32, tag="tp",
                                           name="tpt")
                            ress = s2sb.tile([128, 512], bf16, tag="res",
                                             name="res")
                            for half in range(2):
                                hp = 2 * cc + half
                                sp = s2_ps.tile([128, 512], f32, tag="s2",
                                                name="s2t")
                                nc.tensor.matmul(
                                    sp[:],
                                    a_bd4[cc][:, 128 * half:128 * (half + 1)],
                                    q_sb[cc][:, 512 * nt:512 * (nt + 1)],
                                    start=True, stop=True)
                                e2 = s2sb.tile([128, 512], bf16, tag="e2",
                                               name="e2")
                                nc.scalar.activation(e2[:], sp[:], AF.Exp,
                                                     scale=SCALE)
                                gp = g_ps.tile([128, 272], f32,
                                               tag="g", name="gt")
                                for sub in range(4):
                                    nc.tensor.matmul(
                                        gp[:, 68 * sub:68 * sub + 68],
                                        e2[:, 128 * sub:128 * (sub + 1)],
                                        attnZ[hp], start=True, stop=True,
                                        skip_group_check=True)
                                r2 = s2sb.tile([128, 8], f32, tag="r2",
                                               name="r2")
                                gv = gp[:].rearrange(
                                    "p (s c) -> p s c", s=4, c=68)
                                nc.vector.reciprocal(r2[:], gv[:, :, 64:66])
                                for sub in range(4):
                                    sA = r2[:, 2 * sub:2 * sub + 1]
                                    sB = r2[:, 2 * sub + 1:2 * sub + 2]
                                    inA = gp[:, 68 * sub:68 * sub + 32]
                                    inB = gp[:, 68 * sub + 32:68 * sub + 64]
                                    oA = ress[:, 128 * sub + 64 * half:
                                              128 * sub + 64 * half + 32]
                                    oB = ress[:, 128 * sub + 64 * half + 32:
                                              128 * sub + 64 * half + 64]
                                    if half == 0:
                                        nc.scalar.activation(
                                            oA, inA, AF.Copy, scale=sA)
                                        nc.vector.tensor_scalar_mul(
                                            oB, inB, sB)
                                    else:
                                        nc.vector.tensor_scalar_mul(
                                            oA, inA, sA)
                                        nc.scalar.activation(
                                            oB, inB, AF.Copy, scale=sB)
                            tpb = tp[:].bitcast(bf16)
                            for sub in range(4):
                                nc.tensor.transpose(
                                    tpb[:, 256 * sub:256 * sub + 128],
                                    ress[:, 128 * sub:128 * (sub + 1)],
                                    idn[:])
                            sl = att_out[cc][:, 512 * nt:512 * (nt + 1)]
                            nc.vector.tensor_tensor(
                                sl.rearrange("p (a b) -> p a b", a=4, b=128),
                                tpb.rearrange("p (a b) -> p a b",
                                              a=4, b=256)[:, :, 0:128],
                                sl.rearrange("p (a b) -> p a b", a=4, b=128),
                                ALU.add)
                        for mc in range(2):
                            pp = t_ps.tile([128, 512], f32, tag="tp",
                                           name="prt")
                            for kc in range(2):
                                nc.tensor.matmul(
                                    pp[:],
                                    pwv[:, kc, 128 * mc:128 * (mc + 1)],
                                    att_out[kc][:, 512 * nt:512 * (nt + 1)],
                                    start=(kc == 0), stop=(kc == 1))
                            ot = osb.tile([128, 512], f32, tag="ot",
                                          name="ott")
                            nc.scalar.activation(ot[:], pp[:], AF.Identity,
                                                 bias=pb[:, mc:mc + 1])
                            nc.sync.dma_start(
                                OUT[mc, :, 512 * nt:512 * (nt + 1)], ot[:])

                    for nt in range(7):
                        conv_rb(0, aw[0], nt)
                        conv_rb(1, aw[1], nt)
                        if nt > 0:
                            stage2(nt - 1)
                    conv_rb(0, aw[0], 7)
                    stage2(6)
                    conv_rb(1, aw[1], 7)
                    stage2(7)

    nc.compile()
    return nc


def _prep_consts(qkv_w, qkv_s, qkv_b, pe_w, pe_s, pe_b, proj_w, proj_s,
                 proj_b):
    import ml_dtypes
    bf = ml_dtypes.bfloat16
    f = np.float32
    w = np.asarray(qkv_w, f).copy()          # [768, 256, 3, 3]
    dif = (w[:, :, 0, 1] + w[:, :, 1, 0] + w[:, :, 1, 1] + w[:, :, 1, 2]
           + w[:, :, 2, 1])
    w[:, :, 1, 1] -= THETA * dif
    w *= np.asarray(qkv_s, f)[:, None, None, None]
    # WQ[mc, p, kc, s, o'] = w[128*mc+o', 128*kc+p, s//3, s%3]
    wqf = w.reshape(6, 128, 2, 128, 9)       # [mc, o', kc, p, s]
    wqf = np.ascontiguousarray(
        wqf.transpose(0, 3, 2, 4, 1))        # [6,128,2,9,128] f32
    wq = wqf.astype(bf)
    awf = np.ascontiguousarray(wqf[0:2])     # q-block weights in f32

    bq = np.ascontiguousarray(np.asarray(qkv_b, f).reshape(6, 128).T)

    pe_wf = np.asarray(pe_w, f)[:, 0] * np.asarray(pe_s, f)[:, None, None]
    pew = np.zeros((128, 2, 9), f)
    for kc in range(2):
        for s in range(9):
            pew[:, kc, s] = pe_wf[128 * kc:128 * (kc + 1), s // 3, s % 3]

    pwm = np.asarray(proj_w, f)[:, :, 0, 0] * np.asarray(proj_s, f)[:, None]
    pw = np.ascontiguousarray(
        pwm.T.reshape(2, 128, 256).transpose(1, 0, 2).reshape(
            128, 512)).astype(bf)
    pbv = np.asarray(proj_b, f) + pwm @ np.asarray(pe_b, f)
    pb = np.ascontiguousarray(pbv.reshape(2, 128).T)

    idn = np.eye(128, dtype=bf)
    return dict(wq=wq, awf=awf, bq=bq, pew=pew, pw=pw, pb=pb, idn=idn)


def kernel(x, qkv_w, qkv_s, qkv_b, pe_w, pe_s, pe_b, proj_w, proj_s, proj_b):
    import ml_dtypes
    from concourse.bass_utils import run_bass_kernel_spmd

    if "nc" not in _cache:
        _cache["nc"] = _build()
    nc = _cache["nc"]

    consts = _prep_consts(qkv_w, qkv_s, qkv_b, pe_w, pe_s, pe_b, proj_w,
                          proj_s, proj_b)
    x = np.asarray(x, np.float32).astype(ml_dtypes.bfloat16)
    in_maps = []
    for b in range(B):
        m = dict(consts)
        m["x"] = np.ascontiguousarray(x[b].reshape(2, 128, H, W))
        in_maps.append(m)

    res = run_bass_kernel_spmd(nc, in_maps, list(range(N_CORES)), trace=False)
    out = np.empty((B, C, H, W), np.float32)
    for b in range(B):
        out[b] = res.results[b]["out"].reshape(C, H, W)
    return out


# revision 38
# speedup vs baseline: 1.1603x; 1.1603x over previous
"""Trainium2 Bass kernel for agent attention (sparse_attention problem).

Per-core work (data-parallel over batch B=8 across 8 NeuronCores):
  x[b] [256, 64, 64] -> qkv 3x3 conv (dif-conv + BN folded into weights)
  -> agent attention (8 heads, d=32, 64 agent tokens)
  -> depthwise 3x3 pe conv on v -> 1x1 proj.

Fully pipelined single phase: v conv -> agent tokens computed directly
from 8x8 block sums of x (so stage-1 needs only k) -> k conv with
stage-1 attention interleaved per row-block pair -> q conv with stage-2
+ proj interleaved (lagged one pair). All matmuls bf16 (fp32 PSUM).
"""
import numpy as np

NUM_HEADS = 8
AGENT_NUM = 64
THETA = 0.7
C = 256
H = W = 64
HW = H * W
D = C // NUM_HEADS          # 32
N_AG = AGENT_NUM            # 64
PS = 8                      # pool size
N_CORES = 8
B = 8

_cache = {}


def _build():
    import concourse.bass as bass
    import concourse.tile as tile
    from concourse import bacc, mybir

    f32 = mybir.dt.float32
    f32r = mybir.dt.float32r
    bf16 = mybir.dt.bfloat16
    AF = mybir.ActivationFunctionType
    ALU = mybir.AluOpType
    AX = mybir.AxisListType

    nc = bacc.Bacc("TRN2", target_bir_lowering=False, debug=False,
                   enable_asserts=True, num_devices=N_CORES)

    X = nc.dram_tensor("x", [2, 128, H, W], bf16, kind="ExternalInput").ap()
    WQ = nc.dram_tensor("wq", [6, 128, 2, 9, 128], bf16,
                        kind="ExternalInput").ap()
    AWF = nc.dram_tensor("awf", [2, 128, 2, 9, 128], f32r,
                         kind="ExternalInput").ap()
    BQ = nc.dram_tensor("bq", [128, 6], f32, kind="ExternalInput").ap()
    PEW = nc.dram_tensor("pew", [128, 2, 9], f32, kind="ExternalInput").ap()
    PW = nc.dram_tensor("pw", [128, 2 * 256], bf16, kind="ExternalInput").ap()
    PB = nc.dram_tensor("pb", [128, 2], f32, kind="ExternalInput").ap()
    IDN = nc.dram_tensor("idn", [128, 128], bf16, kind="ExternalInput").ap()
    OUT = nc.dram_tensor("out", [2, 128, HW], f32, kind="ExternalOutput").ap()

    # softmax exp scale: d^-0.5, with the 1/64 agent-pool mean folded in
    SCALE = (D ** -0.5) / (PS * PS)

    with tile.TileContext(nc) as tc:
        from contextlib import ExitStack
        with ExitStack() as top:
            pers = top.enter_context(tc.tile_pool(name="pers", bufs=1))
            q_sb = [pers.tile([128, HW], bf16, tag=f"q{i}", name=f"q{i}")
                    for i in range(2)]
            k_sb = [pers.tile([128, HW], bf16, tag=f"k{i}", name=f"k{i}")
                    for i in range(2)]
            v_pad = [pers.tile([128, 66 * 66], bf16, tag=f"vp{i}",
                               name=f"vp{i}") for i in range(2)]
            att_out = [pers.tile([128, HW], bf16, tag=f"ao{i}", name=f"ao{i}")
                       for i in range(2)]
            bq = pers.tile([128, 6], f32, tag="bq", name="bq")
            idn = pers.tile([128, 128], bf16, tag="idn", name="idn")
            pew = pers.tile([128, 2, 9], f32, tag="pew", name="pew")
            ab64 = pers.tile([128, 2], f32, tag="ab64", name="ab64")
            abd_t = pers.tile([128, 512], bf16, tag="abd", name="abd")
            a_bd4 = [abd_t[:, 256 * i:256 * (i + 1)] for i in range(2)]
            az_t = pers.tile([128, 4 * 68], bf16, tag="az", name="az")
            attnZ = [az_t[:, 68 * i:68 * (i + 1)] for i in range(4)]
            pw = pers.tile([128, 2 * 256], bf16, tag="pw", name="pwt")
            pb = pers.tile([128, 2], f32, tag="pb", name="pbt")
            nc.sync.dma_start(bq[:], BQ[:])
            nc.sync.dma_start(idn[:], IDN[:])
            nc.sync.dma_start(pew[:], PEW[:])
            nc.sync.dma_start(pw[:], PW[:])
            nc.sync.dma_start(pb[:], PB[:])
            pwv = pw[:].rearrange("p (a b) -> p a b", a=2, b=256)

            for cc in range(2):
                vv = v_pad[cc][:].rearrange("p (r c) -> p r c", r=66, c=66)
                nc.vector.memset(vv[:, 0:1, :], 0.0)
                nc.vector.memset(vv[:, 65:66, :], 0.0)
                nc.vector.memset(vv[:, :, 0:1], 0.0)
                nc.vector.memset(vv[:, :, 65:66], 0.0)

            s1sb = top.enter_context(tc.tile_pool(name="s1sb", bufs=2))

            with ExitStack() as ph:
                cpool = ph.enter_context(tc.tile_pool(name="conv", bufs=1))
                wpool = ph.enter_context(tc.tile_pool(name="wq", bufs=1))
                cps = ph.enter_context(
                    tc.tile_pool(name="cps", bufs=2, space="PSUM"))
                pepool = ph.enter_context(tc.tile_pool(name="pep", bufs=2))
                xbpool = ph.enter_context(tc.tile_pool(name="xbp", bufs=2))

                x_pad = [cpool.tile([128, 66 * 66], bf16, tag=f"xp{i}",
                                    name=f"xp{i}") for i in range(2)]
                x_pk = [cpool.tile([128, HW], bf16, tag=f"xk{i}",
                                   name=f"xk{i}") for i in range(2)]
                def load_w(mc):
                    wts = []
                    for kc in range(2):
                        wt = wpool.tile([128, 9, 128], bf16, tag="w",
                                        name="w", bufs=4)
                        nc.sync.dma_start(wt[:], WQ[mc, :, kc])
                        wts.append(wt)
                    return wts

                for kc in range(2):
                    # packed contiguous DMA (few descriptors), then pad
                    # on-chip with fast bf16 copies
                    nc.sync.dma_start(x_pk[kc][:, 0:2048], X[kc, :, 0:32])
                    nc.sync.dma_start(x_pk[kc][:, 2048:4096],
                                      X[kc, :, 32:64])
                    xv = x_pad[kc][:].rearrange(
                        "p (r c) -> p r c", r=66, c=66)
                    nc.vector.memset(xv[:, 0:1, :], 0.0)
                    nc.vector.memset(xv[:, 65:66, :], 0.0)
                    nc.vector.memset(xv[:, :, 0:1], 0.0)
                    nc.vector.memset(xv[:, :, 65:66], 0.0)
                    xkv = x_pk[kc][:].rearrange(
                        "p (r c) -> p r c", r=64, c=64)
                    nc.vector.tensor_copy(xv[:, 1:33, 1:65], xkv[:, 0:32])
                    nc.vector.tensor_copy(xv[:, 33:65, 1:65], xkv[:, 32:64])
                wv = [load_w(4), load_w(5)]

                nc.vector.tensor_scalar_mul(ab64[:], bq[:, 0:2], 64.0)

                # 8x8 block sums of padded x for all 9 conv offsets:
                # xblk[kc][cin, 3*ky+kx, 8*by+bx], kept in f32
                xblk_b = []
                for kc in range(2):
                    xv = x_pad[kc][:].rearrange(
                        "p (r c) -> p r c", r=66, c=66)
                    rs = xbpool.tile([128, 3, 66, 8], f32, tag="rs",
                                     name="rs")
                    for kx in range(3):
                        nc.vector.tensor_reduce(
                            rs[:, kx], xv[:, :, kx:kx + 64].rearrange(
                                "p r (b d) -> p r b d", b=8, d=8),
                            AX.X, ALU.add)
                    xbf = xbpool.tile([128, 9, 64], f32r, tag="xbf",
                                      name="xbf")
                    with nc.allow_low_precision(
                            reason="f32r write of f32-accumulated sums"):
                        for ky in range(3):
                            for kx in range(3):
                                nc.vector.tensor_reduce(
                                    xbf[:, 3 * ky + kx].rearrange(
                                        "p (y x) -> p y x", y=8, x=8),
                                    rs[:, kx, ky:ky + 64, :].rearrange(
                                        "p (b d) x -> p b x d", b=8, d=8),
                                    AX.X, ALU.add)
                    xblk_b.append(xbf)

                def conv_rb(mc, wts, rb):
                    ps_t = cps.tile([128, 512], f32, tag="cps", name="cpst")
                    psv = ps_t[:].rearrange("p (r c) -> p r c", r=8, c=64)
                    i = 0
                    for kc in range(2):
                        xv = x_pad[kc][:].rearrange(
                            "p (r c) -> p r c", r=66, c=66)
                        for s in range(9):
                            ky, kx = s // 3, s % 3
                            rhs = xv[:, 8 * rb + ky: 8 * rb + ky + 8,
                                     kx: kx + 64]
                            nc.tensor.matmul(
                                psv, wts[kc][:, s, :], rhs,
                                start=(i == 0), stop=(i == 17))
                            i += 1
                    bias = bq[:, mc: mc + 1]
                    if mc < 2:
                        dst = q_sb[mc][:, 512 * rb: 512 * (rb + 1)]
                        nc.scalar.activation(dst, ps_t[:], AF.Identity,
                                             bias=bias)
                    elif mc < 4:
                        dst = k_sb[mc - 2][:, 512 * rb: 512 * (rb + 1)]
                        nc.scalar.activation(dst, ps_t[:], AF.Identity,
                                             bias=bias)
                    else:
                        vv = v_pad[mc - 4][:].rearrange(
                            "p (r c) -> p r c", r=66, c=66)
                        dst = vv[:, 8 * rb + 1: 8 * rb + 9, 1:65]
                        nc.scalar.activation(dst, psv, AF.Identity,
                                             bias=bias)

                # pe depthwise conv on DVE in bf16: scaled-copy taps via
                # tensor_scalar_mul (4x mode) + tensor_tensor adds (2x mode)
                def pe_conv(cc):
                    vvf = v_pad[cc][:].rearrange(
                        "p (r c) -> p r c", r=66, c=66)
                    dst = att_out[cc][:].rearrange(
                        "p (r c) -> p r c", r=64, c=64)
                    for s in range(9):
                        ky, kx = s // 3, s % 3
                        sv = vvf[:, ky: ky + 64, kx: kx + 64]
                        if s == 0:
                            nc.vector.tensor_scalar_mul(
                                dst, sv, pew[:, cc, 0:1])
                        else:
                            tmp = pepool.tile([128, HW], bf16, tag="pet",
                                              name="pet")
                            tv = tmp[:].rearrange(
                                "p (r c) -> p r c", r=64, c=64)
                            nc.vector.tensor_scalar_mul(
                                tv, sv, pew[:, cc, s:s + 1])
                            nc.vector.tensor_tensor(dst, tv, dst, ALU.add)

                # ---- v convs + agent tokens + transposed v ----
                vts = [None] * 32
                # q-group weights (bf16 for the conv, f32 for the
                # agent-token matmuls) — DMA'd after the v weights so the
                # first conv isn't queued behind them
                aw = []
                awf = []
                for mc in range(2):
                    awm = []
                    awfm = []
                    for kc in range(2):
                        wt = wpool.tile([128, 9, 128], bf16, tag="aw",
                                        name="aw", bufs=4)
                        nc.sync.dma_start(wt[:], WQ[mc, :, kc])
                        awm.append(wt)
                        wtf = wpool.tile([128, 9, 128], f32r, tag="awf",
                                         name="awf", bufs=4)
                        nc.sync.dma_start(wtf[:], AWF[mc, :, kc])
                        awfm.append(wtf)
                    aw.append(awm)
                    awf.append(awfm)
                for rb in range(8):
                    conv_rb(4, wv[0], rb)
                with ExitStack() as vph:
                    a_pp = vph.enter_context(
                        tc.tile_pool(name="aps", bufs=1, space="PSUM"))
                    tr_ps = vph.enter_context(
                        tc.tile_pool(name="trps", bufs=2, space="PSUM"))
                    a_ps = a_pp.tile([128, 128], f32, tag="ap", name="apt")
                    for mc in range(2):
                        i = 0
                        for kc in range(2):
                            for s in range(9):
                                nc.tensor.matmul(
                                    a_ps[:, 64 * mc:64 * (mc + 1)],
                                    awf[mc][kc][:, s, :],
                                    xblk_b[kc][:, s, :],
                                    start=(i == 0), stop=(i == 17),
                                    skip_group_check=True)
                                i += 1

                    def make_vt(ch):
                        vtc = s1sb.tile([128, 264], bf16, tag="vt",
                                        name="vt", bufs=32)
                        vts[ch] = vtc
                        vtv = vtc[:].rearrange("p (a b) -> p a b", a=4, b=66)
                        nc.vector.memset(vtv[:, :, 64:66], 1.0)
                        for cc in range(2):
                            vv = v_pad[cc][:].rearrange(
                                "p (r c) -> p r c", r=66, c=66)
                            vstg = s1sb.tile([128, 128], bf16, tag="vstg",
                                             name="vstg")
                            nc.vector.tensor_copy(
                                vstg[:].rearrange(
                                    "p (r c) -> p r c", r=2, c=64),
                                vv[:, 2 * ch + 1: 2 * ch + 3, 1:65])
                            tp = tr_ps.tile([128, 128], bf16, tag="tr",
                                            name="trt")
                            nc.tensor.transpose(tp[:], vstg[:], idn[:])
                            nc.vector.tensor_copy(
                                vtc[:].rearrange(
                                    "p (a b) -> p a b", a=4, b=66)[
                                    :, 2 * cc: 2 * cc + 2, 0:64],
                                tp[:].rearrange("p (a b) -> p a b",
                                                a=2, b=64))

                    for rb in range(8):
                        conv_rb(5, wv[1], rb)
                        for ch in range(4 * rb, 4 * rb + 4):
                            make_vt(ch)
                    # agent tokens: block-diag a (+64*bias), bf16
                    for cc in range(2):
                        nc.vector.memset(a_bd4[cc], 0.0)
                        for j in range(4):
                            nc.vector.tensor_scalar_add(
                                a_bd4[cc][32 * j:32 * j + 32,
                                          64 * j:64 * j + 64],
                                a_ps[32 * j:32 * j + 32,
                                     64 * cc:64 * (cc + 1)],
                                ab64[32 * j:32 * j + 32, cc:cc + 1])
                pe_conv(0)
                pe_conv(1)

                # ---- k convs + stage 1 ----
                with ExitStack() as s1ph:
                    st_ps = s1ph.enter_context(
                        tc.tile_pool(name="stps", bufs=2, space="PSUM"))
                    at_pp = s1ph.enter_context(
                        tc.tile_pool(name="atps", bufs=1, space="PSUM"))
                    # interleaved long-lived accumulation groups must each
                    # own a PSUM bank
                    attn_ps = [at_pp.tile([128, 66], f32, tag=f"at{i}",
                                          name=f"at{i}")[:] for i in range(4)]

                    ets = [None] * 32

                    def s1_sp(ch):
                        sp = st_ps.tile([128, 512], f32, tag="st",
                                        name="stt")
                        for cc in range(2):
                            nc.tensor.matmul(
                                sp[:, 256 * cc:256 * (cc + 1)],
                                k_sb[cc][:, 128 * ch:128 * (ch + 1)],
                                a_bd4[cc][:], start=True, stop=True,
                                skip_group_check=True)
                        et = s1sb.tile([128, 512], bf16, tag="et", name="et",
                                       bufs=4)
                        nc.scalar.activation(et[:], sp[:], AF.Exp,
                                             scale=SCALE)
                        ets[ch] = et

                    def s1_agg(ch):
                        for hp in range(4):
                            nc.tensor.matmul(
                                attn_ps[hp],
                                ets[ch][:, 128 * hp:128 * (hp + 1)],
                                vts[ch][:, 66 * hp:66 * hp + 66],
                                start=(ch == 0), stop=(ch == 31))

                    wk = [load_w(2), load_w(3)]
                    for r in range(8):
                        conv_rb(2, wk[0], r)
                        conv_rb(3, wk[1], r)
                        for ch in range(4 * r, 4 * r + 4):
                            s1_sp(ch)
                            if ch > 0:
                                s1_agg(ch - 1)
                    s1_agg(31)

                    # normalize stage-1 rows by Z1, build attnZ (+ones cols)
                    for hp in range(4):
                        r1 = s1sb.tile([128, 1], f32, tag="r1", name="r1")
                        nc.vector.reciprocal(r1[:], attn_ps[hp][:, 64:65])
                        nc.vector.memset(attnZ[hp], 0.0)
                        nc.vector.memset(attnZ[hp][0:64, 64:65], 1.0)
                        nc.vector.memset(attnZ[hp][64:128, 65:66], 1.0)
                        nc.vector.tensor_scalar_mul(
                            attnZ[hp][0:64, 0:32], attn_ps[hp][0:64, 0:32],
                            r1[0:64, :])
                        nc.vector.tensor_scalar_mul(
                            attnZ[hp][64:128, 32:64],
                            attn_ps[hp][64:128, 32:64], r1[64:128, :])

                # ---- q convs + stage 2 + proj ----
                with ExitStack() as s2ph:
                    s2sb = s2ph.enter_context(
                        tc.tile_pool(name="s2sb", bufs=3))
                    osb = s2ph.enter_context(tc.tile_pool(name="osb",
                                                          bufs=3))
                    s2_ps = s2ph.enter_context(
                        tc.tile_pool(name="s2ps", bufs=2, space="PSUM"))
                    g_ps = s2ph.enter_context(
                        tc.tile_pool(name="gps", bufs=2, space="PSUM"))
                    t_ps = s2ph.enter_context(
                        tc.tile_pool(name="tps", bufs=2, space="PSUM"))

                    def stage2(nt):
                        for cc in range(2):
                            tp = t_ps.tile([128, 512], f32, tag="tp",
                                           name="tpt")
                            ress = s2sb.tile([128, 512], bf16, tag="res",
                                             name="res")
                            for half in range(2):
                                hp = 2 * cc + half
                                sp = s2_ps.tile([128, 512], f32, tag="s2",
                                                name="s2t")
                                nc.tensor.matmul(
                                    sp[:],
                                    a_bd4[cc][:, 128 * half:128 * (half + 1)],
                                    q_sb[cc][:, 512 * nt:512 * (nt + 1)],
                                    start=True, stop=True)
                                e2 = s2sb.tile([128, 512], bf16, tag="e2",
                                               name="e2")
                                nc.scalar.activation(e2[:], sp[:], AF.Exp,
                                                     scale=SCALE)
                                gp = g_ps.tile([128, 272], f32,
                                               tag="g", name="gt")
                                for sub in range(4):
                                    nc.tensor.matmul(
                                        gp[:, 68 * sub:68 * sub + 68],
                                        e2[:, 128 * sub:128 * (sub + 1)],
                                        attnZ[hp], start=True, stop=True,
                                        skip_group_check=True)
                                r2 = s2sb.tile([128, 8], f32, tag="r2",
                                               name="r2")
                                gv = gp[:].rearrange(
                                    "p (s c) -> p s c", s=4, c=68)
                                nc.vector.reciprocal(r2[:], gv[:, :, 64:66])
                                for sub in range(4):
                                    sA = r2[:, 2 * sub:2 * sub + 1]
                                    sB = r2[:, 2 * sub + 1:2 * sub + 2]
                                    inA = gp[:, 68 * sub:68 * sub + 32]
                                    inB = gp[:, 68 * sub + 32:68 * sub + 64]
                                    oA = ress[:, 128 * sub + 64 * half:
                                              128 * sub + 64 * half + 32]
                                    oB = ress[:, 128 * sub + 64 * half + 32:
                                              128 * sub + 64 * half + 64]
                                    if half == 0:
                                        nc.scalar.activation(
                                            oA, inA, AF.Copy, scale=sA)
                                        nc.vector.tensor_scalar_mul(
                                            oB, inB, sB)
                                    else:
                                        nc.vector.tensor_scalar_mul(
                                            oA, inA, sA)
                                        nc.scalar.activation(
                                            oB, inB, AF.Copy, scale=sB)
                            tpb = tp[:].bitcast(bf16)
                            for sub in range(4):
                                nc.tensor.transpose(
                                    tpb[:, 256 * sub:256 * sub + 128],
                                    ress[:, 128 * sub:128 * (sub + 1)],
                                    idn[:])
                            sl = att_out[cc][:, 512 * nt:512 * (nt + 1)]
                            nc.vector.tensor_tensor(
                                sl.rearrange("p (a b) -> p a b", a=4, b=128),
                                tpb.rearrange("p (a b) -> p a b",
                                              a=4, b=256)[:, :, 0:128],
                                sl.rearrange("p (a b) -> p a b", a=4, b=128),
                                ALU.add)
                        for mc in range(2):
                            pp = t_ps.tile([128, 512], f32, tag="tp",
                                           name="prt")
                            for kc in range(2):
                                nc.tensor.matmul(
                                    pp[:],
                                    pwv[:, kc, 128 * mc:128 * (mc + 1)],
                                    att_out[kc][:, 512 * nt:512 * (nt + 1)],
                                    start=(kc == 0), stop=(kc == 1))
                            ot = osb.tile([128, 512], f32, tag="ot",
                                          name="ott")
                            nc.scalar.activation(ot[:], pp[:], AF.Identity,
                                                 bias=pb[:, mc:mc + 1])
                            nc.sync.dma_start(
                                OUT[mc, :, 512 * nt:512 * (nt + 1)], ot[:])

                    for nt in range(7):
                        conv_rb(0, aw[0], nt)
                        conv_rb(1, aw[1], nt)
                        if nt > 0:
                            stage2(nt - 1)
                    conv_rb(0, aw[0], 7)
                    stage2(6)
                    conv_rb(1, aw[1], 7)
                    stage2(7)

    nc.compile()
    return nc


def _prep_consts(qkv_w, qkv_s, qkv_b, pe_w, pe_s, pe_b, proj_w, proj_s,
                 proj_b):
    import ml_dtypes
    bf = ml_dtypes.bfloat16
    f = np.float32
    w = np.asarray(qkv_w, f).copy()          # [768, 256, 3, 3]
    dif = (w[:, :, 0, 1] + w[:, :, 1, 0] + w[:, :, 1, 1] + w[:, :, 1, 2]
           + w[:, :, 2, 1])
    w[:, :, 1, 1] -= THETA * dif
    w *= np.asarray(qkv_s, f)[:, None, None, None]
    # WQ[mc, p, kc, s, o'] = w[128*mc+o', 128*kc+p, s//3, s%3]
    wqf = w.reshape(6, 128, 2, 128, 9)       # [mc, o', kc, p, s]
    wqf = np.ascontiguousarray(
        wqf.transpose(0, 3, 2, 4, 1))        # [6,128,2,9,128] f32
    wq = wqf.astype(bf)
    awf = np.ascontiguousarray(wqf[0:2])     # q-block weights in f32

    bq = np.ascontiguousarray(np.asarray(qkv_b, f).reshape(6, 128).T)

    pe_wf = np.asarray(pe_w, f)[:, 0] * np.asarray(pe_s, f)[:, None, None]
    pew = np.zeros((128, 2, 9), f)
    for kc in range(2):
        for s in range(9):
            pew[:, kc, s] = pe_wf[128 * kc:128 * (kc + 1), s // 3, s % 3]

    pwm = np.asarray(proj_w, f)[:, :, 0, 0] * np.asarray(proj_s, f)[:, None]
    pw = np.ascontiguousarray(
        pwm.T.reshape(2, 128, 256).transpose(1, 0, 2).reshape(
            128, 512)).astype(bf)
    pbv = np.asarray(proj_b, f) + pwm @ np.asarray(pe_b, f)
    pb = np.ascontiguousarray(pbv.reshape(2, 128).T)

    idn = np.eye(128, dtype=bf)
    return dict(wq=wq, awf=awf, bq=bq, pew=pew, pw=pw, pb=pb, idn=idn)


def kernel(x, qkv_w, qkv_s, qkv_b, pe_w, pe_s, pe_b, proj_w, proj_s, proj_b):
    import ml_dtypes
    from concourse.bass_utils import run_bass_kernel_spmd

    if "nc" not in _cache:
        _cache["nc"] = _build()
    nc = _cache["nc"]

    consts = _prep_consts(qkv_w, qkv_s, qkv_b, pe_w, pe_s, pe_b, proj_w,
                          proj_s, proj_b)
    x = np.asarray(x, np.float32).astype(ml_dtypes.bfloat16)
    in_maps = []
    for b in range(B):
        m = dict(consts)
        m["x"] = np.ascontiguousarray(x[b].reshape(2, 128, H, W))
        in_maps.append(m)

    res = run_bass_kernel_spmd(nc, in_maps, list(range(N_CORES)), trace=False)
    out = np.empty((B, C, H, W), np.float32)
    for b in range(B):
        out[b] = res.results[b]["out"].reshape(C, H, W)
    return out


# revision 39
# speedup vs baseline: 1.1677x; 1.0064x over previous
"""Trainium2 Bass kernel for agent attention (sparse_attention problem).

Per-core work (data-parallel over batch B=8 across 8 NeuronCores):
  x[b] [256, 64, 64] -> qkv 3x3 conv (dif-conv + BN folded into weights)
  -> agent attention (8 heads, d=32, 64 agent tokens)
  -> depthwise 3x3 pe conv on v -> 1x1 proj.

Fully pipelined single phase: v conv -> agent tokens computed directly
from 8x8 block sums of x (so stage-1 needs only k) -> k conv with
stage-1 attention interleaved per row-block pair -> q conv with stage-2
+ proj interleaved (lagged one pair). All matmuls bf16 (fp32 PSUM).
"""
import numpy as np

NUM_HEADS = 8
AGENT_NUM = 64
THETA = 0.7
C = 256
H = W = 64
HW = H * W
D = C // NUM_HEADS          # 32
N_AG = AGENT_NUM            # 64
PS = 8                      # pool size
N_CORES = 8
B = 8

_cache = {}


def _build():
    import concourse.bass as bass
    import concourse.tile as tile
    from concourse import bacc, mybir

    f32 = mybir.dt.float32
    f32r = mybir.dt.float32r
    bf16 = mybir.dt.bfloat16
    AF = mybir.ActivationFunctionType
    ALU = mybir.AluOpType
    AX = mybir.AxisListType

    nc = bacc.Bacc("TRN2", target_bir_lowering=False, debug=False,
                   enable_asserts=True, num_devices=N_CORES)

    X = nc.dram_tensor("x", [2, 128, H, W], bf16, kind="ExternalInput").ap()
    WQ = nc.dram_tensor("wq", [6, 128, 2, 9, 128], bf16,
                        kind="ExternalInput").ap()
    AWF = nc.dram_tensor("awf", [2, 128, 2, 9, 128], f32r,
                         kind="ExternalInput").ap()
    BQ = nc.dram_tensor("bq", [128, 6], f32, kind="ExternalInput").ap()
    PEW = nc.dram_tensor("pew", [128, 2, 9], f32, kind="ExternalInput").ap()
    PW = nc.dram_tensor("pw", [128, 2 * 256], bf16, kind="ExternalInput").ap()
    PB = nc.dram_tensor("pb", [128, 2], f32, kind="ExternalInput").ap()
    IDN = nc.dram_tensor("idn", [128, 128], bf16, kind="ExternalInput").ap()
    OUT = nc.dram_tensor("out", [2, 128, HW], f32, kind="ExternalOutput").ap()

    # softmax exp scale: d^-0.5, with the 1/64 agent-pool mean folded in
    SCALE = (D ** -0.5) / (PS * PS)

    with tile.TileContext(nc) as tc:
        from contextlib import ExitStack
        with ExitStack() as top:
            pers = top.enter_context(tc.tile_pool(name="pers", bufs=1))
            q_sb = [pers.tile([128, HW], bf16, tag=f"q{i}", name=f"q{i}")
                    for i in range(2)]
            k_sb = [pers.tile([128, HW], bf16, tag=f"k{i}", name=f"k{i}")
                    for i in range(2)]
            v_pad = [pers.tile([128, 66 * 66], bf16, tag=f"vp{i}",
                               name=f"vp{i}") for i in range(2)]
            att_out = [pers.tile([128, HW], bf16, tag=f"ao{i}", name=f"ao{i}")
                       for i in range(2)]
            bq = pers.tile([128, 6], f32, tag="bq", name="bq")
            idn = pers.tile([128, 128], bf16, tag="idn", name="idn")
            pew = pers.tile([128, 2, 9], f32, tag="pew", name="pew")
            ab64 = pers.tile([128, 2], f32, tag="ab64", name="ab64")
            abd_t = pers.tile([128, 512], bf16, tag="abd", name="abd")
            a_bd4 = [abd_t[:, 256 * i:256 * (i + 1)] for i in range(2)]
            az_t = pers.tile([128, 4 * 68], bf16, tag="az", name="az")
            attnZ = [az_t[:, 68 * i:68 * (i + 1)] for i in range(4)]
            pw = pers.tile([128, 2 * 256], bf16, tag="pw", name="pwt")
            pb = pers.tile([128, 2], f32, tag="pb", name="pbt")
            nc.sync.dma_start(bq[:], BQ[:])
            nc.sync.dma_start(idn[:], IDN[:])
            nc.sync.dma_start(pew[:], PEW[:])
            nc.sync.dma_start(pw[:], PW[:])
            nc.sync.dma_start(pb[:], PB[:])
            pwv = pw[:].rearrange("p (a b) -> p a b", a=2, b=256)

            for cc in range(2):
                vv = v_pad[cc][:].rearrange("p (r c) -> p r c", r=66, c=66)
                nc.vector.memset(vv[:, 0:1, :], 0.0)
                nc.vector.memset(vv[:, 65:66, :], 0.0)
                nc.vector.memset(vv[:, :, 0:1], 0.0)
                nc.vector.memset(vv[:, :, 65:66], 0.0)

            s1sb = top.enter_context(tc.tile_pool(name="s1sb", bufs=2))

            with ExitStack() as ph:
                cpool = ph.enter_context(tc.tile_pool(name="conv", bufs=1))
                wpool = ph.enter_context(tc.tile_pool(name="wq", bufs=1))
                cps = ph.enter_context(
                    tc.tile_pool(name="cps", bufs=2, space="PSUM"))
                pepool = ph.enter_context(tc.tile_pool(name="pep", bufs=2))
                xbpool = ph.enter_context(tc.tile_pool(name="xbp", bufs=2))

                x_pad = [cpool.tile([128, 66 * 66], bf16, tag=f"xp{i}",
                                    name=f"xp{i}") for i in range(2)]
                x_pk = [cpool.tile([128, HW], bf16, tag=f"xk{i}",
                                   name=f"xk{i}") for i in range(2)]
                def load_w(mc):
                    wts = []
                    for kc in range(2):
                        wt = wpool.tile([128, 9, 128], bf16, tag="w",
                                        name="w", bufs=4)
                        nc.sync.dma_start(wt[:], WQ[mc, :, kc])
                        wts.append(wt)
                    return wts

                wv = [load_w(4), load_w(5)]
                for kc in range(2):
                    # packed contiguous DMA (few descriptors), then pad
                    # on-chip with fast bf16 copies
                    nc.sync.dma_start(x_pk[kc][:, 0:2048], X[kc, :, 0:32])
                    nc.sync.dma_start(x_pk[kc][:, 2048:4096],
                                      X[kc, :, 32:64])
                    xv = x_pad[kc][:].rearrange(
                        "p (r c) -> p r c", r=66, c=66)
                    nc.vector.memset(xv[:, 0:1, :], 0.0)
                    nc.vector.memset(xv[:, 65:66, :], 0.0)
                    nc.vector.memset(xv[:, :, 0:1], 0.0)
                    nc.vector.memset(xv[:, :, 65:66], 0.0)
                    xkv = x_pk[kc][:].rearrange(
                        "p (r c) -> p r c", r=64, c=64)
                    nc.vector.tensor_copy(xv[:, 1:33, 1:65], xkv[:, 0:32])
                    nc.vector.tensor_copy(xv[:, 33:65, 1:65], xkv[:, 32:64])

                nc.vector.tensor_scalar_mul(ab64[:], bq[:, 0:2], 64.0)

                # 8x8 block sums of padded x for all 9 conv offsets:
                # xblk[kc][cin, 3*ky+kx, 8*by+bx], kept in f32
                xblk_b = []
                for kc in range(2):
                    xv = x_pad[kc][:].rearrange(
                        "p (r c) -> p r c", r=66, c=66)
                    rs = xbpool.tile([128, 3, 66, 8], f32, tag="rs",
                                     name="rs")
                    for kx in range(3):
                        nc.vector.tensor_reduce(
                            rs[:, kx], xv[:, :, kx:kx + 64].rearrange(
                                "p r (b d) -> p r b d", b=8, d=8),
                            AX.X, ALU.add)
                    xbf = xbpool.tile([128, 9, 64], f32r, tag="xbf",
                                      name="xbf")
                    with nc.allow_low_precision(
                            reason="f32r write of f32-accumulated sums"):
                        for ky in range(3):
                            for kx in range(3):
                                nc.vector.tensor_reduce(
                                    xbf[:, 3 * ky + kx].rearrange(
                                        "p (y x) -> p y x", y=8, x=8),
                                    rs[:, kx, ky:ky + 64, :].rearrange(
                                        "p (b d) x -> p b x d", b=8, d=8),
                                    AX.X, ALU.add)
                    xblk_b.append(xbf)

                def conv_rb(mc, wts, rb):
                    ps_t = cps.tile([128, 512], f32, tag="cps", name="cpst")
                    psv = ps_t[:].rearrange("p (r c) -> p r c", r=8, c=64)
                    i = 0
                    for kc in range(2):
                        xv = x_pad[kc][:].rearrange(
                            "p (r c) -> p r c", r=66, c=66)
                        for s in range(9):
                            ky, kx = s // 3, s % 3
                            rhs = xv[:, 8 * rb + ky: 8 * rb + ky + 8,
                                     kx: kx + 64]
                            nc.tensor.matmul(
                                psv, wts[kc][:, s, :], rhs,
                                start=(i == 0), stop=(i == 17))
                            i += 1
                    bias = bq[:, mc: mc + 1]
                    if mc < 2:
                        dst = q_sb[mc][:, 512 * rb: 512 * (rb + 1)]
                        nc.scalar.activation(dst, ps_t[:], AF.Identity,
                                             bias=bias)
                    elif mc < 4:
                        dst = k_sb[mc - 2][:, 512 * rb: 512 * (rb + 1)]
                        nc.scalar.activation(dst, ps_t[:], AF.Identity,
                                             bias=bias)
                    else:
                        vv = v_pad[mc - 4][:].rearrange(
                            "p (r c) -> p r c", r=66, c=66)
                        dst = vv[:, 8 * rb + 1: 8 * rb + 9, 1:65]
                        nc.scalar.activation(dst, psv, AF.Identity,
                                             bias=bias)

                # pe depthwise conv on DVE in bf16: scaled-copy taps via
                # tensor_scalar_mul (4x mode) + tensor_tensor adds (2x mode)
                def pe_conv(cc):
                    vvf = v_pad[cc][:].rearrange(
                        "p (r c) -> p r c", r=66, c=66)
                    dst = att_out[cc][:].rearrange(
                        "p (r c) -> p r c", r=64, c=64)
                    for s in range(9):
                        ky, kx = s // 3, s % 3
                        sv = vvf[:, ky: ky + 64, kx: kx + 64]
                        if s == 0:
                            nc.vector.tensor_scalar_mul(
                                dst, sv, pew[:, cc, 0:1])
                        else:
                            tmp = pepool.tile([128, HW], bf16, tag="pet",
                                              name="pet")
                            tv = tmp[:].rearrange(
                                "p (r c) -> p r c", r=64, c=64)
                            nc.vector.tensor_scalar_mul(
                                tv, sv, pew[:, cc, s:s + 1])
                            nc.vector.tensor_tensor(dst, tv, dst, ALU.add)

                # ---- v convs + agent tokens + transposed v ----
                vts = [None] * 32
                # q-group weights (bf16 for the conv, f32 for the
                # agent-token matmuls) — DMA'd after the v weights so the
                # first conv isn't queued behind them
                aw = []
                awf = []
                for mc in range(2):
                    awm = []
                    awfm = []
                    for kc in range(2):
                        wt = wpool.tile([128, 9, 128], bf16, tag="aw",
                                        name="aw", bufs=4)
                        nc.sync.dma_start(wt[:], WQ[mc, :, kc])
                        awm.append(wt)
                        wtf = wpool.tile([128, 9, 128], f32r, tag="awf",
                                         name="awf", bufs=4)
                        nc.sync.dma_start(wtf[:], AWF[mc, :, kc])
                        awfm.append(wtf)
                    aw.append(awm)
                    awf.append(awfm)
                for rb in range(8):
                    conv_rb(4, wv[0], rb)
                with ExitStack() as vph:
                    a_pp = vph.enter_context(
                        tc.tile_pool(name="aps", bufs=1, space="PSUM"))
                    tr_ps = vph.enter_context(
                        tc.tile_pool(name="trps", bufs=2, space="PSUM"))
                    a_ps = a_pp.tile([128, 128], f32, tag="ap", name="apt")
                    for mc in range(2):
                        i = 0
                        for kc in range(2):
                            for s in range(9):
                                nc.tensor.matmul(
                                    a_ps[:, 64 * mc:64 * (mc + 1)],
                                    awf[mc][kc][:, s, :],
                                    xblk_b[kc][:, s, :],
                                    start=(i == 0), stop=(i == 17),
                                    skip_group_check=True)
                                i += 1

                    def make_vt(ch):
                        vtc = s1sb.tile([128, 264], bf16, tag="vt",
                                        name="vt", bufs=32)
                        vts[ch] = vtc
                        vtv = vtc[:].rearrange("p (a b) -> p a b", a=4, b=66)
                        nc.vector.memset(vtv[:, :, 64:66], 1.0)
                        for cc in range(2):
                            vv = v_pad[cc][:].rearrange(
                                "p (r c) -> p r c", r=66, c=66)
                            vstg = s1sb.tile([128, 128], bf16, tag="vstg",
                                             name="vstg")
                            nc.vector.tensor_copy(
                                vstg[:].rearrange(
                                    "p (r c) -> p r c", r=2, c=64),
                                vv[:, 2 * ch + 1: 2 * ch + 3, 1:65])
                            tp = tr_ps.tile([128, 128], bf16, tag="tr",
                                            name="trt")
                            nc.tensor.transpose(tp[:], vstg[:], idn[:])
                            nc.vector.tensor_copy(
                                vtc[:].rearrange(
                                    "p (a b) -> p a b", a=4, b=66)[
                                    :, 2 * cc: 2 * cc + 2, 0:64],
                                tp[:].rearrange("p (a b) -> p a b",
                                                a=2, b=64))

                    for rb in range(8):
                        conv_rb(5, wv[1], rb)
                        for ch in range(4 * rb, 4 * rb + 4):
                            make_vt(ch)
                    # agent tokens: block-diag a (+64*bias), bf16
                    for cc in range(2):
                        nc.vector.memset(a_bd4[cc], 0.0)
                        for j in range(4):
                            nc.vector.tensor_scalar_add(
                                a_bd4[cc][32 * j:32 * j + 32,
                                          64 * j:64 * j + 64],
                                a_ps[32 * j:32 * j + 32,
                                     64 * cc:64 * (cc + 1)],
                                ab64[32 * j:32 * j + 32, cc:cc + 1])
                pe_conv(0)
                pe_conv(1)

                # ---- k convs + stage 1 ----
                with ExitStack() as s1ph:
                    st_ps = s1ph.enter_context(
                        tc.tile_pool(name="stps", bufs=2, space="PSUM"))
                    at_pp = s1ph.enter_context(
                        tc.tile_pool(name="atps", bufs=1, space="PSUM"))
                    # interleaved long-lived accumulation groups must each
                    # own a PSUM bank
                    attn_ps = [at_pp.tile([128, 66], f32, tag=f"at{i}",
                                          name=f"at{i}")[:] for i in range(4)]

                    ets = [None] * 32

                    def s1_sp(ch):
                        sp = st_ps.tile([128, 512], f32, tag="st",
                                        name="stt")
                        for cc in range(2):
                            nc.tensor.matmul(
                                sp[:, 256 * cc:256 * (cc + 1)],
                                k_sb[cc][:, 128 * ch:128 * (ch + 1)],
                                a_bd4[cc][:], start=True, stop=True,
                                skip_group_check=True)
                        et = s1sb.tile([128, 512], bf16, tag="et", name="et",
                                       bufs=4)
                        nc.scalar.activation(et[:], sp[:], AF.Exp,
                                             scale=SCALE)
                        ets[ch] = et

                    def s1_agg(ch):
                        for hp in range(4):
                            nc.tensor.matmul(
                                attn_ps[hp],
                                ets[ch][:, 128 * hp:128 * (hp + 1)],
                                vts[ch][:, 66 * hp:66 * hp + 66],
                                start=(ch == 0), stop=(ch == 31))

                    wk = [load_w(2), load_w(3)]
                    for r in range(8):
                        conv_rb(2, wk[0], r)
                        conv_rb(3, wk[1], r)
                        for ch in range(4 * r, 4 * r + 4):
                            s1_sp(ch)
                            if ch > 0:
                                s1_agg(ch - 1)
                    s1_agg(31)

                    # normalize stage-1 rows by Z1, build attnZ (+ones cols)
                    for hp in range(4):
                        r1 = s1sb.tile([128, 1], f32, tag="r1", name="r1")
                        nc.vector.reciprocal(r1[:], attn_ps[hp][:, 64:65])
                        nc.vector.memset(attnZ[hp], 0.0)
                        nc.vector.memset(attnZ[hp][0:64, 64:65], 1.0)
                        nc.vector.memset(attnZ[hp][64:128, 65:66], 1.0)
                        nc.vector.tensor_scalar_mul(
                            attnZ[hp][0:64, 0:32], attn_ps[hp][0:64, 0:32],
                            r1[0:64, :])
                        nc.vector.tensor_scalar_mul(
                            attnZ[hp][64:128, 32:64],
                            attn_ps[hp][64:128, 32:64], r1[64:128, :])

                # ---- q convs + stage 2 + proj ----
                with ExitStack() as s2ph:
                    s2sb = s2ph.enter_context(
                        tc.tile_pool(name="s2sb", bufs=3))
                    osb = s2ph.enter_context(tc.tile_pool(name="osb",
                                                          bufs=3))
                    s2_ps = s2ph.enter_context(
                        tc.tile_pool(name="s2ps", bufs=2, space="PSUM"))
                    g_ps = s2ph.enter_context(
                        tc.tile_pool(name="gps", bufs=2, space="PSUM"))
                    t_ps = s2ph.enter_context(
                        tc.tile_pool(name="tps", bufs=2, space="PSUM"))

                    def stage2(nt):
                        for cc in range(2):
                            tp = t_ps.tile([128, 512], f32, tag="tp",
                                           name="tpt")
                            ress = s2sb.tile([128, 512], bf16, tag="res",
                                             name="res")
                            for half in range(2):
                                hp = 2 * cc + half
                                sp = s2_ps.tile([128, 512], f32, tag="s2",
                                                name="s2t")
                                nc.tensor.matmul(
                                    sp[:],
                                    a_bd4[cc][:, 128 * half:128 * (half + 1)],
                                    q_sb[cc][:, 512 * nt:512 * (nt + 1)],
                                    start=True, stop=True)
                                e2 = s2sb.tile([128, 512], bf16, tag="e2",
                                               name="e2")
                                nc.scalar.activation(e2[:], sp[:], AF.Exp,
                                                     scale=SCALE)
                                gp = g_ps.tile([128, 272], f32,
                                               tag="g", name="gt")
                                for sub in range(4):
                                    nc.tensor.matmul(
                                        gp[:, 68 * sub:68 * sub + 68],
                                        e2[:, 128 * sub:128 * (sub + 1)],
                                        attnZ[hp], start=True, stop=True,
                                        skip_group_check=True)
                                r2 = s2sb.tile([128, 8], f32, tag="r2",
                                               name="r2")
                                gv = gp[:].rearrange(
                                    "p (s c) -> p s c", s=4, c=68)
                                nc.vector.reciprocal(r2[:], gv[:, :, 64:66])
                                for sub in range(4):
                                    sA = r2[:, 2 * sub:2 * sub + 1]
                                    sB = r2[:, 2 * sub + 1:2 * sub + 2]
                                    inA = gp[:, 68 * sub:68 * sub + 32]
                                    inB = gp[:, 68 * sub + 32:68 * sub + 64]
                                    oA = ress[:, 128 * sub + 64 * half:
                                              128 * sub + 64 * half + 32]
                                    oB = ress[:, 128 * sub + 64 * half + 32:
                                              128 * sub + 64 * half + 64]
                                    if half == 0:
                                        nc.scalar.activation(
                                            oA, inA, AF.Copy, scale=sA)
                                        nc.vector.tensor_scalar_mul(
                                            oB, inB, sB)
                                    else:
                                        nc.vector.tensor_scalar_mul(
                                            oA, inA, sA)
                                        nc.scalar.activation(
                                            oB, inB, AF.Copy, scale=sB)
                            tpb = tp[:].bitcast(bf16)
                            for sub in range(4):
                                nc.tensor.transpose(
                                    tpb[:, 256 * sub:256 * sub + 128],
                                    ress[:, 128 * sub:128 * (sub + 1)],
                                    idn[:])
                            sl = att_out[cc][:, 512 * nt:512 * (nt + 1)]
                            nc.vector.tensor_tensor(
                                sl.rearrange("p (a b) -> p a b", a=4, b=128),
                                tpb.rearrange("p (a b) -> p a b",
                                              a=4, b=256)[:, :, 0:128],
                                sl.rearrange("p (a b) -> p a b", a=4, b=128),
                                ALU.add)
                        for mc in range(2):
                            pp = t_ps.tile([128, 512], f32, tag="tp",
                                           name="prt")
                            for kc in range(2):
                                nc.tensor.matmul(
                                    pp[:],
                                    pwv[:, kc, 128 * mc:128 * (mc + 1)],
                                    att_out[kc][:, 512 * nt:512 * (nt + 1)],
                                    start=(kc == 0), stop=(kc == 1))
                            ot = osb.tile([128, 512], f32, tag="ot",
                                          name="ott")
                            nc.scalar.activation(ot[:], pp[:], AF.Identity,
                                                 bias=pb[:, mc:mc + 1])
                            nc.sync.dma_start(
                                OUT[mc, :, 512 * nt:512 * (nt + 1)], ot[:])

                    for nt in range(7):
                        conv_rb(0, aw[0], nt)
                        conv_rb(1, aw[1], nt)
                        if nt > 0:
                            stage2(nt - 1)
                    conv_rb(0, aw[0], 7)
                    stage2(6)
                    conv_rb(1, aw[1], 7)
                    stage2(7)

    nc.compile()
    return nc


def _prep_consts(qkv_w, qkv_s, qkv_b, pe_w, pe_s, pe_b, proj_w, proj_s,
                 proj_b):
    import ml_dtypes
    bf = ml_dtypes.bfloat16
    f = np.float32
    w = np.asarray(qkv_w, f).copy()          # [768, 256, 3, 3]
    dif = (w[:, :, 0, 1] + w[:, :, 1, 0] + w[:, :, 1, 1] + w[:, :, 1, 2]
           + w[:, :, 2, 1])
    w[:, :, 1, 1] -= THETA * dif
    w *= np.asarray(qkv_s, f)[:, None, None, None]
    # WQ[mc, p, kc, s, o'] = w[128*mc+o', 128*kc+p, s//3, s%3]
    wqf = w.reshape(6, 128, 2, 128, 9)       # [mc, o', kc, p, s]
    wqf = np.ascontiguousarray(
        wqf.transpose(0, 3, 2, 4, 1))        # [6,128,2,9,128] f32
    wq = wqf.astype(bf)
    awf = np.ascontiguousarray(wqf[0:2])     # q-block weights in f32

    bq = np.ascontiguousarray(np.asarray(qkv_b, f).reshape(6, 128).T)

    pe_wf = np.asarray(pe_w, f)[:, 0] * np.asarray(pe_s, f)[:, None, None]
    pew = np.zeros((128, 2, 9), f)
    for kc in range(2):
        for s in range(9):
            pew[:, kc, s] = pe_wf[128 * kc:128 * (kc + 1), s // 3, s % 3]

    pwm = np.asarray(proj_w, f)[:, :, 0, 0] * np.asarray(proj_s, f)[:, None]
    pw = np.ascontiguousarray(
        pwm.T.reshape(2, 128, 256).transpose(1, 0, 2).reshape(
            128, 512)).astype(bf)
    pbv = np.asarray(proj_b, f) + pwm @ np.asarray(pe_b, f)
    pb = np.ascontiguousarray(pbv.reshape(2, 128).T)

    idn = np.eye(128, dtype=bf)
    return dict(wq=wq, awf=awf, bq=bq, pew=pew, pw=pw, pb=pb, idn=idn)


def kernel(x, qkv_w, qkv_s, qkv_b, pe_w, pe_s, pe_b, proj_w, proj_s, proj_b):
    import ml_dtypes
    from concourse.bass_utils import run_bass_kernel_spmd

    if "nc" not in _cache:
        _cache["nc"] = _build()
    nc = _cache["nc"]

    consts = _prep_consts(qkv_w, qkv_s, qkv_b, pe_w, pe_s, pe_b, proj_w,
                          proj_s, proj_b)
    x = np.asarray(x, np.float32).astype(ml_dtypes.bfloat16)
    in_maps = []
    for b in range(B):
        m = dict(consts)
        m["x"] = np.ascontiguousarray(x[b].reshape(2, 128, H, W))
        in_maps.append(m)

    res = run_bass_kernel_spmd(nc, in_maps, list(range(N_CORES)), trace=False)
    out = np.empty((B, C, H, W), np.float32)
    for b in range(B):
        out[b] = res.results[b]["out"].reshape(C, H, W)
    return out


# revision 42
# speedup vs baseline: 1.1732x; 1.0047x over previous
"""Trainium2 Bass kernel for agent attention (sparse_attention problem).

Per-core work (data-parallel over batch B=8 across 8 NeuronCores):
  x[b] [256, 64, 64] -> qkv 3x3 conv (dif-conv + BN folded into weights)
  -> agent attention (8 heads, d=32, 64 agent tokens)
  -> depthwise 3x3 pe conv on v -> 1x1 proj.

Fully pipelined single phase: v conv -> agent tokens computed directly
from 8x8 block sums of x (so stage-1 needs only k) -> k conv with
stage-1 attention interleaved per row-block pair -> q conv with stage-2
+ proj interleaved (lagged one pair). All matmuls bf16 (fp32 PSUM).
"""
import numpy as np

NUM_HEADS = 8
AGENT_NUM = 64
THETA = 0.7
C = 256
H = W = 64
HW = H * W
D = C // NUM_HEADS          # 32
N_AG = AGENT_NUM            # 64
PS = 8                      # pool size
N_CORES = 8
B = 8

_cache = {}


def _build():
    import concourse.bass as bass
    import concourse.tile as tile
    from concourse import bacc, mybir

    f32 = mybir.dt.float32
    f32r = mybir.dt.float32r
    bf16 = mybir.dt.bfloat16
    AF = mybir.ActivationFunctionType
    ALU = mybir.AluOpType
    AX = mybir.AxisListType

    nc = bacc.Bacc("TRN2", target_bir_lowering=False, debug=False,
                   enable_asserts=True, num_devices=N_CORES)

    X = nc.dram_tensor("x", [2, 128, H, W], bf16, kind="ExternalInput").ap()
    WQ = nc.dram_tensor("wq", [6, 128, 2, 9, 128], bf16,
                        kind="ExternalInput").ap()
    AWF = nc.dram_tensor("awf", [2, 128, 2, 9, 128], f32r,
                         kind="ExternalInput").ap()
    BQ = nc.dram_tensor("bq", [128, 6], f32, kind="ExternalInput").ap()
    PEW = nc.dram_tensor("pew", [128, 2, 9], f32, kind="ExternalInput").ap()
    PW = nc.dram_tensor("pw", [128, 2 * 256], bf16, kind="ExternalInput").ap()
    PB = nc.dram_tensor("pb", [128, 2], f32, kind="ExternalInput").ap()
    IDN = nc.dram_tensor("idn", [128, 128], bf16, kind="ExternalInput").ap()
    OUT = nc.dram_tensor("out", [2, 128, HW], f32, kind="ExternalOutput").ap()

    # softmax exp scale: d^-0.5, with the 1/64 agent-pool mean folded in
    SCALE = (D ** -0.5) / (PS * PS)

    with tile.TileContext(nc) as tc:
        from contextlib import ExitStack
        with ExitStack() as top:
            pers = top.enter_context(tc.tile_pool(name="pers", bufs=1))
            q_sb = [pers.tile([128, HW], bf16, tag=f"q{i}", name=f"q{i}")
                    for i in range(2)]
            k_sb = [pers.tile([128, HW], bf16, tag=f"k{i}", name=f"k{i}")
                    for i in range(2)]
            v_pad = [pers.tile([128, 66 * 66], bf16, tag=f"vp{i}",
                               name=f"vp{i}") for i in range(2)]
            att_out = [pers.tile([128, HW], bf16, tag=f"ao{i}", name=f"ao{i}")
                       for i in range(2)]
            bq = pers.tile([128, 6], f32, tag="bq", name="bq")
            idn = pers.tile([128, 128], bf16, tag="idn", name="idn")
            pew = pers.tile([128, 2, 9], f32, tag="pew", name="pew")
            ab64 = pers.tile([128, 2], f32, tag="ab64", name="ab64")
            abd_t = pers.tile([128, 512], bf16, tag="abd", name="abd")
            a_bd4 = [abd_t[:, 256 * i:256 * (i + 1)] for i in range(2)]
            az_t = pers.tile([128, 4 * 68], bf16, tag="az", name="az")
            attnZ = [az_t[:, 68 * i:68 * (i + 1)] for i in range(4)]
            pw = pers.tile([128, 2 * 256], bf16, tag="pw", name="pwt")
            pb = pers.tile([128, 2], f32, tag="pb", name="pbt")
            nc.sync.dma_start(bq[:], BQ[:])
            nc.sync.dma_start(idn[:], IDN[:])
            nc.sync.dma_start(pew[:], PEW[:])
            nc.sync.dma_start(pw[:], PW[:])
            nc.sync.dma_start(pb[:], PB[:])
            pwv = pw[:].rearrange("p (a b) -> p a b", a=2, b=256)

            for cc in range(2):
                vv = v_pad[cc][:].rearrange("p (r c) -> p r c", r=66, c=66)
                nc.vector.memset(vv[:, 0:1, :], 0.0)
                nc.vector.memset(vv[:, 65:66, :], 0.0)
                nc.vector.memset(vv[:, :, 0:1], 0.0)
                nc.vector.memset(vv[:, :, 65:66], 0.0)

            s1sb = top.enter_context(tc.tile_pool(name="s1sb", bufs=2))

            with ExitStack() as ph:
                cpool = ph.enter_context(tc.tile_pool(name="conv", bufs=1))
                wpool = ph.enter_context(tc.tile_pool(name="wq", bufs=1))
                cps = ph.enter_context(
                    tc.tile_pool(name="cps", bufs=2, space="PSUM"))
                pepool = ph.enter_context(tc.tile_pool(name="pep", bufs=2))
                xbpool = ph.enter_context(tc.tile_pool(name="xbp", bufs=2))

                x_pad = [cpool.tile([128, 66 * 66], bf16, tag=f"xp{i}",
                                    name=f"xp{i}") for i in range(2)]
                x_pk = [cpool.tile([128, HW], bf16, tag=f"xk{i}",
                                   name=f"xk{i}") for i in range(2)]
                def load_w(mc):
                    wts = []
                    for kc in range(2):
                        wt = wpool.tile([128, 9, 128], bf16, tag="w",
                                        name="w", bufs=4)
                        nc.sync.dma_start(wt[:], WQ[mc, :, kc])
                        wts.append(wt)
                    return wts

                wv = [load_w(4), load_w(5)]
                # packed contiguous DMA (few descriptors), first chunks of
                # both kc halves first, then pad on-chip with bf16 copies
                nc.sync.dma_start(x_pk[0][:, 0:2048], X[0, :, 0:32])
                nc.sync.dma_start(x_pk[1][:, 0:2048], X[1, :, 0:32])
                nc.sync.dma_start(x_pk[0][:, 2048:4096], X[0, :, 32:64])
                nc.sync.dma_start(x_pk[1][:, 2048:4096], X[1, :, 32:64])
                for kc in range(2):
                    xv = x_pad[kc][:].rearrange(
                        "p (r c) -> p r c", r=66, c=66)
                    nc.vector.memset(xv[:, 0:1, :], 0.0)
                    nc.vector.memset(xv[:, 65:66, :], 0.0)
                    nc.vector.memset(xv[:, :, 0:1], 0.0)
                    nc.vector.memset(xv[:, :, 65:66], 0.0)
                    xkv = x_pk[kc][:].rearrange(
                        "p (r c) -> p r c", r=64, c=64)
                    nc.vector.tensor_copy(xv[:, 1:33, 1:65], xkv[:, 0:32])
                    nc.vector.tensor_copy(xv[:, 33:65, 1:65], xkv[:, 32:64])

                nc.vector.tensor_scalar_mul(ab64[:], bq[:, 0:2], 64.0)

                # 8x8 block sums of padded x for all 9 conv offsets:
                # xblk[kc][cin, 3*ky+kx, 8*by+bx], kept in f32
                xblk_b = []
                for kc in range(2):
                    xv = x_pad[kc][:].rearrange(
                        "p (r c) -> p r c", r=66, c=66)
                    rs = xbpool.tile([128, 3, 66, 8], f32, tag="rs",
                                     name="rs")
                    # row sums: reduce once for kx=1, then slide the
                    # 8-wide window left/right with 2 add/sub ops each
                    nc.vector.tensor_reduce(
                        rs[:, 1], xv[:, :, 1:65].rearrange(
                            "p r (b d) -> p r b d", b=8, d=8),
                        AX.X, ALU.add)
                    xg0 = xv[:, :, 0:64].rearrange(
                        "p r (b d) -> p r b d", b=8, d=8)
                    xg1 = xv[:, :, 1:65].rearrange(
                        "p r (b d) -> p r b d", b=8, d=8)
                    xg2 = xv[:, :, 2:66].rearrange(
                        "p r (b d) -> p r b d", b=8, d=8)
                    nc.vector.tensor_tensor(
                        rs[:, 0], rs[:, 1], xg1[:, :, :, 7], ALU.subtract)
                    nc.vector.tensor_tensor(
                        rs[:, 0], rs[:, 0], xg0[:, :, :, 0], ALU.add)
                    nc.vector.tensor_tensor(
                        rs[:, 2], rs[:, 1], xg1[:, :, :, 0], ALU.subtract)
                    nc.vector.tensor_tensor(
                        rs[:, 2], rs[:, 2], xg2[:, :, :, 7], ALU.add)
                    xbf = xbpool.tile([128, 9, 64], f32r, tag="xbf",
                                      name="xbf")
                    with nc.allow_low_precision(
                            reason="f32r write of f32-accumulated sums"):
                        for ky in range(3):
                            for kx in range(3):
                                nc.vector.tensor_reduce(
                                    xbf[:, 3 * ky + kx].rearrange(
                                        "p (y x) -> p y x", y=8, x=8),
                                    rs[:, kx, ky:ky + 64, :].rearrange(
                                        "p (b d) x -> p b x d", b=8, d=8),
                                    AX.X, ALU.add)
                    xblk_b.append(xbf)

                def conv_rb(mc, wts, rb):
                    ps_t = cps.tile([128, 512], f32, tag="cps", name="cpst")
                    psv = ps_t[:].rearrange("p (r c) -> p r c", r=8, c=64)
                    i = 0
                    for kc in range(2):
                        xv = x_pad[kc][:].rearrange(
                            "p (r c) -> p r c", r=66, c=66)
                        for s in range(9):
                            ky, kx = s // 3, s % 3
                            rhs = xv[:, 8 * rb + ky: 8 * rb + ky + 8,
                                     kx: kx + 64]
                            nc.tensor.matmul(
                                psv, wts[kc][:, s, :], rhs,
                                start=(i == 0), stop=(i == 17))
                            i += 1
                    bias = bq[:, mc: mc + 1]
                    if mc < 2:
                        dst = q_sb[mc][:, 512 * rb: 512 * (rb + 1)]
                        nc.scalar.activation(dst, ps_t[:], AF.Identity,
                                             bias=bias)
                    elif mc < 4:
                        dst = k_sb[mc - 2][:, 512 * rb: 512 * (rb + 1)]
                        nc.scalar.activation(dst, ps_t[:], AF.Identity,
                                             bias=bias)
                    else:
                        vv = v_pad[mc - 4][:].rearrange(
                            "p (r c) -> p r c", r=66, c=66)
                        dst = vv[:, 8 * rb + 1: 8 * rb + 9, 1:65]
                        nc.scalar.activation(dst, psv, AF.Identity,
                                             bias=bias)

                # pe depthwise conv on DVE in bf16: scaled-copy taps via
                # tensor_scalar_mul (4x mode) + tensor_tensor adds (2x mode)
                def pe_conv(cc):
                    vvf = v_pad[cc][:].rearrange(
                        "p (r c) -> p r c", r=66, c=66)
                    dst = att_out[cc][:].rearrange(
                        "p (r c) -> p r c", r=64, c=64)
                    for s in range(9):
                        ky, kx = s // 3, s % 3
                        sv = vvf[:, ky: ky + 64, kx: kx + 64]
                        if s == 0:
                            nc.vector.tensor_scalar_mul(
                                dst, sv, pew[:, cc, 0:1])
                        else:
                            tmp = pepool.tile([128, HW], bf16, tag="pet",
                                              name="pet")
                            tv = tmp[:].rearrange(
                                "p (r c) -> p r c", r=64, c=64)
                            nc.vector.tensor_scalar_mul(
                                tv, sv, pew[:, cc, s:s + 1])
                            nc.vector.tensor_tensor(dst, tv, dst, ALU.add)

                # ---- v convs + agent tokens + transposed v ----
                vts = [None] * 32
                # q-group weights (bf16 for the conv, f32 for the
                # agent-token matmuls) — DMA'd after the v weights so the
                # first conv isn't queued behind them
                aw = []
                awf = []
                for mc in range(2):
                    awm = []
                    awfm = []
                    for kc in range(2):
                        wt = wpool.tile([128, 9, 128], bf16, tag="aw",
                                        name="aw", bufs=4)
                        nc.sync.dma_start(wt[:], WQ[mc, :, kc])
                        awm.append(wt)
                        wtf = wpool.tile([128, 9, 128], f32r, tag="awf",
                                         name="awf", bufs=4)
                        nc.sync.dma_start(wtf[:], AWF[mc, :, kc])
                        awfm.append(wtf)
                    aw.append(awm)
                    awf.append(awfm)
                for rb in range(8):
                    conv_rb(4, wv[0], rb)
                with ExitStack() as vph:
                    a_pp = vph.enter_context(
                        tc.tile_pool(name="aps", bufs=1, space="PSUM"))
                    tr_ps = vph.enter_context(
                        tc.tile_pool(name="trps", bufs=2, space="PSUM"))
                    a_ps = a_pp.tile([128, 128], f32, tag="ap", name="apt")
                    for mc in range(2):
                        i = 0
                        for kc in range(2):
                            for s in range(9):
                                nc.tensor.matmul(
                                    a_ps[:, 64 * mc:64 * (mc + 1)],
                                    awf[mc][kc][:, s, :],
                                    xblk_b[kc][:, s, :],
                                    start=(i == 0), stop=(i == 17),
                                    skip_group_check=True)
                                i += 1

                    def make_vt(ch):
                        vtc = s1sb.tile([128, 264], bf16, tag="vt",
                                        name="vt", bufs=32)
                        vts[ch] = vtc
                        vtv = vtc[:].rearrange("p (a b) -> p a b", a=4, b=66)
                        nc.vector.memset(vtv[:, :, 64:66], 1.0)
                        for cc in range(2):
                            vv = v_pad[cc][:].rearrange(
                                "p (r c) -> p r c", r=66, c=66)
                            vstg = s1sb.tile([128, 128], bf16, tag="vstg",
                                             name="vstg")
                            nc.vector.tensor_copy(
                                vstg[:].rearrange(
                                    "p (r c) -> p r c", r=2, c=64),
                                vv[:, 2 * ch + 1: 2 * ch + 3, 1:65])
                            tp = tr_ps.tile([128, 128], bf16, tag="tr",
                                            name="trt")
                            nc.tensor.transpose(tp[:], vstg[:], idn[:])
                            nc.vector.tensor_copy(
                                vtc[:].rearrange(
                                    "p (a b) -> p a b", a=4, b=66)[
                                    :, 2 * cc: 2 * cc + 2, 0:64],
                                tp[:].rearrange("p (a b) -> p a b",
                                                a=2, b=64))

                    for rb in range(8):
                        conv_rb(5, wv[1], rb)
                        for ch in range(4 * rb, 4 * rb + 4):
                            make_vt(ch)
                    # agent tokens: block-diag a (+64*bias), bf16
                    for cc in range(2):
                        nc.vector.memset(a_bd4[cc], 0.0)
                        for j in range(4):
                            nc.vector.tensor_scalar_add(
                                a_bd4[cc][32 * j:32 * j + 32,
                                          64 * j:64 * j + 64],
                                a_ps[32 * j:32 * j + 32,
                                     64 * cc:64 * (cc + 1)],
                                ab64[32 * j:32 * j + 32, cc:cc + 1])
                pe_conv(0)
                pe_conv(1)

                # ---- k convs + stage 1 ----
                with ExitStack() as s1ph:
                    st_ps = s1ph.enter_context(
                        tc.tile_pool(name="stps", bufs=2, space="PSUM"))
                    at_pp = s1ph.enter_context(
                        tc.tile_pool(name="atps", bufs=1, space="PSUM"))
                    # interleaved long-lived accumulation groups must each
                    # own a PSUM bank
                    attn_ps = [at_pp.tile([128, 66], f32, tag=f"at{i}",
                                          name=f"at{i}")[:] for i in range(4)]

                    ets = [None] * 32

                    def s1_sp(ch):
                        sp = st_ps.tile([128, 512], f32, tag="st",
                                        name="stt")
                        for cc in range(2):
                            nc.tensor.matmul(
                                sp[:, 256 * cc:256 * (cc + 1)],
                                k_sb[cc][:, 128 * ch:128 * (ch + 1)],
                                a_bd4[cc][:], start=True, stop=True,
                                skip_group_check=True)
                        et = s1sb.tile([128, 512], bf16, tag="et", name="et",
                                       bufs=4)
                        nc.scalar.activation(et[:], sp[:], AF.Exp,
                                             scale=SCALE)
                        ets[ch] = et

                    def s1_agg(ch):
                        for hp in range(4):
                            nc.tensor.matmul(
                                attn_ps[hp],
                                ets[ch][:, 128 * hp:128 * (hp + 1)],
                                vts[ch][:, 66 * hp:66 * hp + 66],
                                start=(ch == 0), stop=(ch == 31))

                    wk = [load_w(2), load_w(3)]
                    for r in range(8):
                        conv_rb(2, wk[0], r)
                        conv_rb(3, wk[1], r)
                        for ch in range(4 * r, 4 * r + 4):
                            s1_sp(ch)
                            if ch > 0:
                                s1_agg(ch - 1)
                    s1_agg(31)

                    # normalize stage-1 rows by Z1, build attnZ (+ones cols)
                    for hp in range(4):
                        r1 = s1sb.tile([128, 1], f32, tag="r1", name="r1")
                        nc.vector.reciprocal(r1[:], attn_ps[hp][:, 64:65])
                        nc.vector.memset(attnZ[hp], 0.0)
                        nc.vector.memset(attnZ[hp][0:64, 64:65], 1.0)
                        nc.vector.memset(attnZ[hp][64:128, 65:66], 1.0)
                        nc.vector.tensor_scalar_mul(
                            attnZ[hp][0:64, 0:32], attn_ps[hp][0:64, 0:32],
                            r1[0:64, :])
                        nc.vector.tensor_scalar_mul(
                            attnZ[hp][64:128, 32:64],
                            attn_ps[hp][64:128, 32:64], r1[64:128, :])

                # ---- q convs + stage 2 + proj ----
                with ExitStack() as s2ph:
                    s2sb = s2ph.enter_context(
                        tc.tile_pool(name="s2sb", bufs=3))
                    osb = s2ph.enter_context(tc.tile_pool(name="osb",
                                                          bufs=3))
                    s2_ps = s2ph.enter_context(
                        tc.tile_pool(name="s2ps", bufs=2, space="PSUM"))
                    g_ps = s2ph.enter_context(
                        tc.tile_pool(name="gps", bufs=2, space="PSUM"))
                    t_ps = s2ph.enter_context(
                        tc.tile_pool(name="tps", bufs=2, space="PSUM"))

                    def stage2(nt):
                        for cc in range(2):
                            tp = t_ps.tile([128, 512], f32, tag="tp",
                                           name="tpt")
                            ress = s2sb.tile([128, 512], bf16, tag="res",
                                             name="res")
                            for half in range(2):
                                hp = 2 * cc + half
                                sp = s2_ps.tile([128, 512], f32, tag="s2",
                                                name="s2t")
                                nc.tensor.matmul(
                                    sp[:],
                                    a_bd4[cc][:, 128 * half:128 * (half + 1)],
                                    q_sb[cc][:, 512 * nt:512 * (nt + 1)],
                                    start=True, stop=True)
                                e2 = s2sb.tile([128, 512], bf16, tag="e2",
                                               name="e2")
                                nc.scalar.activation(e2[:], sp[:], AF.Exp,
                                                     scale=SCALE)
                                gp = g_ps.tile([128, 272], f32,
                                               tag="g", name="gt")
                                for sub in range(4):
                                    nc.tensor.matmul(
                                        gp[:, 68 * sub:68 * sub + 68],
                                        e2[:, 128 * sub:128 * (sub + 1)],
                                        attnZ[hp], start=True, stop=True,
                                        skip_group_check=True)
                                r2 = s2sb.tile([128, 8], f32, tag="r2",
                                               name="r2")
                                gv = gp[:].rearrange(
                                    "p (s c) -> p s c", s=4, c=68)
                                nc.vector.reciprocal(r2[:], gv[:, :, 64:66])
                                for sub in range(4):
                                    sA = r2[:, 2 * sub:2 * sub + 1]
                                    sB = r2[:, 2 * sub + 1:2 * sub + 2]
                                    inA = gp[:, 68 * sub:68 * sub + 32]
                                    inB = gp[:, 68 * sub + 32:68 * sub + 64]
                                    oA = ress[:, 128 * sub + 64 * half:
                                              128 * sub + 64 * half + 32]
                                    oB = ress[:, 128 * sub + 64 * half + 32:
                                              128 * sub + 64 * half + 64]
                                    if half == 0:
                                        nc.scalar.activation(
                                            oA, inA, AF.Copy, scale=sA)
                                        nc.vector.tensor_scalar_mul(
                                            oB, inB, sB)
                                    else:
                                        nc.vector.tensor_scalar_mul(
                                            oA, inA, sA)
                                        nc.scalar.activation(
                                            oB, inB, AF.Copy, scale=sB)
                            tpb = tp[:].bitcast(bf16)
                            for sub in range(4):
                                nc.tensor.transpose(
                                    tpb[:, 256 * sub:256 * sub + 128],
                                    ress[:, 128 * sub:128 * (sub + 1)],
                                    idn[:])
                            sl = att_out[cc][:, 512 * nt:512 * (nt + 1)]
                            nc.vector.tensor_tensor(
                                sl.rearrange("p (a b) -> p a b", a=4, b=128),
                                tpb.rearrange("p (a b) -> p a b",
                                              a=4, b=256)[:, :, 0:128],
                                sl.rearrange("p (a b) -> p a b", a=4, b=128),
                                ALU.add)
                        for mc in range(2):
                            pp = t_ps.tile([128, 512], f32, tag="tp",
                                           name="prt")
                            for kc in range(2):
                                nc.tensor.matmul(
                                    pp[:],
                                    pwv[:, kc, 128 * mc:128 * (mc + 1)],
                                    att_out[kc][:, 512 * nt:512 * (nt + 1)],
                                    start=(kc == 0), stop=(kc == 1))
                            ot = osb.tile([128, 512], f32, tag="ot",
                                          name="ott")
                            nc.scalar.activation(ot[:], pp[:], AF.Identity,
                                                 bias=pb[:, mc:mc + 1])
                            nc.sync.dma_start(
                                OUT[mc, :, 512 * nt:512 * (nt + 1)], ot[:])

                    for nt in range(7):
                        conv_rb(0, aw[0], nt)
                        conv_rb(1, aw[1], nt)
                        if nt > 0:
                            stage2(nt - 1)
                    conv_rb(0, aw[0], 7)
                    stage2(6)
                    conv_rb(1, aw[1], 7)
                    stage2(7)

    nc.compile()
    return nc


def _prep_consts(qkv_w, qkv_s, qkv_b, pe_w, pe_s, pe_b, proj_w, proj_s,
                 proj_b):
    import ml_dtypes
    bf = ml_dtypes.bfloat16
    f = np.float32
    w = np.asarray(qkv_w, f).copy()          # [768, 256, 3, 3]
    dif = (w[:, :, 0, 1] + w[:, :, 1, 0] + w[:, :, 1, 1] + w[:, :, 1, 2]
           + w[:, :, 2, 1])
    w[:, :, 1, 1] -= THETA * dif
    w *= np.asarray(qkv_s, f)[:, None, None, None]
    # WQ[mc, p, kc, s, o'] = w[128*mc+o', 128*kc+p, s//3, s%3]
    wqf = w.reshape(6, 128, 2, 128, 9)       # [mc, o', kc, p, s]
    wqf = np.ascontiguousarray(
        wqf.transpose(0, 3, 2, 4, 1))        # [6,128,2,9,128] f32
    wq = wqf.astype(bf)
    awf = np.ascontiguousarray(wqf[0:2])     # q-block weights in f32

    bq = np.ascontiguousarray(np.asarray(qkv_b, f).reshape(6, 128).T)

    pe_wf = np.asarray(pe_w, f)[:, 0] * np.asarray(pe_s, f)[:, None, None]
    pew = np.zeros((128, 2, 9), f)
    for kc in range(2):
        for s in range(9):
            pew[:, kc, s] = pe_wf[128 * kc:128 * (kc + 1), s // 3, s % 3]

    pwm = np.asarray(proj_w, f)[:, :, 0, 0] * np.asarray(proj_s, f)[:, None]
    pw = np.ascontiguousarray(
        pwm.T.reshape(2, 128, 256).transpose(1, 0, 2).reshape(
            128, 512)).astype(bf)
    pbv = np.asarray(proj_b, f) + pwm @ np.asarray(pe_b, f)
    pb = np.ascontiguousarray(pbv.reshape(2, 128).T)

    idn = np.eye(128, dtype=bf)
    return dict(wq=wq, awf=awf, bq=bq, pew=pew, pw=pw, pb=pb, idn=idn)


def kernel(x, qkv_w, qkv_s, qkv_b, pe_w, pe_s, pe_b, proj_w, proj_s, proj_b):
    import ml_dtypes
    from concourse.bass_utils import run_bass_kernel_spmd

    if "nc" not in _cache:
        _cache["nc"] = _build()
    nc = _cache["nc"]

    consts = _prep_consts(qkv_w, qkv_s, qkv_b, pe_w, pe_s, pe_b, proj_w,
                          proj_s, proj_b)
    x = np.asarray(x, np.float32).astype(ml_dtypes.bfloat16)
    in_maps = []
    for b in range(B):
        m = dict(consts)
        m["x"] = np.ascontiguousarray(x[b].reshape(2, 128, H, W))
        in_maps.append(m)

    res = run_bass_kernel_spmd(nc, in_maps, list(range(N_CORES)), trace=False)
    out = np.empty((B, C, H, W), np.float32)
    for b in range(B):
        out[b] = res.results[b]["out"].reshape(C, H, W)
    return out
